# revision 28
# baseline (speedup 1.0000x reference)
"""Averaged Hausdorff loss kernel for 8 Trainium2 NeuronCores.

Math: d2[i,j] = |s1_i|^2 + |s2_j|^2 - 2<s1_i, s2_j> computed as a single
K=7 matmul with augmented operands.  Coordinates are rounded to fp16 and
the norm rows hold the hi/lo-split norm OF THE ROUNDED POINTS, so the
matmul produces the exact squared distance between the rounded points
(fp32 PSUM accumulation of exact fp16 products).  Rounding moves each
point by <~8e-4, which shifts the final mean by <~5e-4 relative — far
inside the accuracy gate.  min_j sqrt(d2) = sqrt(min_j d2), so only
per-block row-mins leave the device; sqrt + mean (or max) run on host.

Banding: both sets are sorted by radius r = |x| on host.  By the reverse
triangle inequality, d(i,j) >= |r_i - r_j|, so a query block only needs to
scan candidates whose radius rank falls in a window around its own, plus
the global large-r tail (isolated points live at large r).  Each 128-row
block scans C gathered candidates: a centered rank-window of C - T plus
the T largest-radius points.  After the run the host checks a certificate
per point: found_min + slack <= radius-gap to the nearest excluded
candidate (slack covers the fp16 rounding).

Adaptivity: C per rank-octile is a compile-time "plan".  A narrow plan
(C=1536 uniform) covers low-NN-distance data; a wider per-octile profile
covers independent-draw data.  A cheap host sample of exact NN distances
picks the plan; the post-run certificate escalates narrow -> wide ->
exact-numpy if ever violated, so the result is correct for ANY input.

Layout: global rank-block g (64 blocks of 128 sorted rows) runs on core
g%8, local slot g//8, so all cores share one SPMD program while each
rank-octile (slot) gets its own static width.  Each block's 4·cq-column
quads run as 4 concurrent matmuls on PE row-groups 0/32/64/96 (operand
stacks replicated at those base partitions), two into a kept PSUM tile,
two into a PSUM tile that ScalarE copies to SBUF; a fused DVE MIN2 scan
then consumes kept-PSUM + SBUF copies at 2 elements/cycle.  All operands
for a phase share one wide SBUF tile so each partition row loads in a
single DMA packet (DMA cost here is per-packet, not per-byte)."""

import sys

sys.path.insert(0, "/opt/trn_rl_repo")

import numpy as np

N_CORES = 8
N = 8192          # set1 rows
M = 8192          # set2 rows
D = 3
ROWS_PER_CORE = N // N_CORES          # 1024
BLOCKS = ROWS_PER_CORE // 128         # 8 slots of 128 rows per core
N_BLOCKS = N // 128                   # 64 global rank-blocks
T_TAIL = 128                          # global large-radius tail columns
K = 7                                 # augmented contraction dim
FP32_MAX = 3.4e38
GROUPS = (0, 32, 64, 96)              # PE row-group base partitions

# per-slot candidate counts (C); slot s serves global blocks g with g//8==s
PLAN_NARROW = (768,) * 8
PLAN_WIDE = (1536, 1536, 2048, 2048, 2048, 2560, 2560, 1536)

# certificate-failing points get their exact NN recomputed on host, up to
# this many per direction (1% of the points); beyond that, escalate plans
PATCH_QUOTA = 164

_programs = {}


def _quads(C):
    """Decompose a block's C columns into quads of 4 concurrent chunks."""
    out = [512] * (C // 2048)
    if C % 2048:
        out.append((C % 2048) // 4)
    return out


def _register_min2():
    """Register a custom fused DVE op (per-NEFF uop table): a running
    min-scan over min(in0[p,k], in1[p,k]) seeded from s0.  The output is
    written through a stride-0 broadcast AP, so the single destination cell
    ends up holding the total min.  One instruction consumes two fp32
    streams (PSUM + SBUF ports) at 2 elements/cycle — twice the throughput
    of stock tensor_reduce, with no separate accumulator step."""
    from concourse import dve_ops
    from concourse.dve_spec import Spec, Src0, Src1, minn, C0, AluOp, lower, scan
    from concourse.dve_spec import _has_src1 as has_src1
    from concourse.dve_uop import DveOpSpec

    def _ref(in0, in1, c0, c1, c2):
        b = np.minimum(in0.astype(np.float32), in1.astype(np.float32))
        P = b.shape[0]
        init = (
            np.asarray(c0, np.float32).reshape(-1, 1)
            if np.ndim(c0)
            else np.full((P, 1), c0, np.float32)
        )
        flat = np.minimum.accumulate(
            np.concatenate([init, b.reshape(P, -1)], axis=1), axis=-1
        )[:, 1:]
        return flat.reshape(b.shape)

    ops_out = []
    for name, spec in (
        (
            "MINSCAN_ANT",
            Spec(body=scan(AluOp.MIN, minn(Src0, Src1), init=C0),
                 reference=_ref),
        ),
    ):
        if name in dve_ops._SUB_OPCODE_FOR_NAME:
            ops_out.append(next(op for op in dve_ops.OPS if op.name == name))
            continue
        op = dve_ops.DveOp(name, spec, subdim=False, uops_sha={})
        dve_ops.OPS.append(op)
        dve_ops._SUB_OPCODE_FOR_NAME[name] = (
            dve_ops._CUSTOM_DVE_ROW_BASE + len(dve_ops.OPS) - 1
        )
        assert dve_ops._SUB_OPCODE_FOR_NAME[name] < 0x20
        dve_ops.CUSTOM_DVE_SPECS[name] = spec
        for ver in ("v3", "v4"):
            compiled = DveOpSpec(
                name=name,
                opcode=dve_ops.get_dve_sub_opcode(name),
                uops=lower(spec, ver=ver),
                rd1_en=has_src1(spec),
            )
            op.uops_sha[ver] = compiled.sha(ver)
        ops_out.append(op)
    return ops_out[0]


def _build_program(plan):
    import concourse.tile as tile
    from concourse import bacc, mybir

    min2 = _register_min2()

    nc = bacc.Bacc("TRN2", target_bir_lowering=False, debug=False)
    f32 = mybir.dt.float32
    f16 = mybir.dt.float16

    KR = 96 + K   # operand stack height: replicas at partitions 0/32/64/96
    gw = [c // 4 for c in plan]          # per-slot per-group column count
    goff = np.cumsum([0] + gw)           # per-slot offset within a group
    GW = int(goff[-1])                   # total per-group columns
    E1 = int(goff[1])                    # block 0 rides the early tile
    E2 = int(goff[4])                    # blocks 1-3 on mid1, 4-7 on mid2
    L = ROWS_PER_CORE                    # 1024 lhs columns

    # four phase tensors; each partition row carries all operands needed
    # in that phase, so one DMA per (phase, row-group) = K packets
    early_d = nc.dram_tensor("early", [4 * K, L + E1], f16, kind="ExternalInput")
    mid1_d = nc.dram_tensor("mid1", [4 * K, E2 - E1], f16, kind="ExternalInput")
    mid2_d = nc.dram_tensor("mid2", [4 * K, GW - E2], f16, kind="ExternalInput")
    late_d = nc.dram_tensor("late", [4 * K, L + GW], f16, kind="ExternalInput")
    out_d = nc.dram_tensor("out", [128, 2 * BLOCKS], f32, kind="ExternalOutput")

    with tile.TileContext(nc) as tc:
        with (
            tc.tile_pool(name="ops", bufs=1) as ops,
            tc.tile_pool(name="ps_keep", bufs=2, space="PSUM") as ps_keep,
            tc.tile_pool(name="ps_copy", bufs=2, space="PSUM") as ps_copy,
            tc.tile_pool(name="scopy", bufs=4) as scopy,
            tc.tile_pool(name="small", bufs=1) as small,
        ):
            t_early = ops.tile([KR, L + E1], f16, tag="t_early")
            t_mid1 = ops.tile([KR, E2 - E1], f16, tag="t_mid1")
            t_mid2 = ops.tile([KR, GW - E2], f16, tag="t_mid2")
            t_late = ops.tile([KR, L + GW], f16, tag="t_late")

            # 16 DMAs total (4 phases x 4 row-groups), split across the
            # sync and gpsimd queues; ScalarE issues none (DMA triggers
            # would queue ahead of its copies and stall the pipeline).
            queues = (nc.sync, nc.gpsimd)
            qi = 0
            for t, src in (
                (t_early, early_d),
                (t_mid1, mid1_d),
                (t_mid2, mid2_d),
                (t_late, late_d),
            ):
                for i, g in enumerate(GROUPS):
                    queues[qi % 2].dma_start(
                        t[g : g + K, :], src[i * K : (i + 1) * K, :]
                    )
                    qi += 1

            rowmin = small.tile([128, 2 * BLOCKS], f32, tag="rowmin")

            # Per quad: 4 matmuls on row-groups 0/32/64/96 (concurrent —
            # distinct groups and distinct PSUM banks), ScalarE copies the
            # pc half to SBUF, fused MIN2 scan consumes pk (PSUM port) +
            # sc (SBUF port) at 2 elements/cycle into the block's row-min
            # cell via a stride-0 broadcast AP.
            for o in (0, 1):
                lhs_s = t_early if o == 0 else t_late
                for b in range(BLOCKS):
                    ob = o * BLOCKS + b
                    bc = slice(b * 128, (b + 1) * 128)
                    if o == 0:
                        if b < 1:
                            rt, t0 = t_early, L + int(goff[b])
                        elif b < 4:
                            rt, t0 = t_mid1, int(goff[b]) - E1
                        else:
                            rt, t0 = t_mid2, int(goff[b]) - E2
                    else:
                        rt = t_late
                        t0 = L + int(goff[b])
                    for qi_, cq in enumerate(_quads(plan[b])):
                        pk = ps_keep.tile([128, 2, 512], f32, name="pk", tag="pk")
                        pc = ps_copy.tile([128, 2, 512], f32, name="pc", tag="pc")
                        for i, g in enumerate(GROUPS):
                            dst = (pk, pc)[i % 2]
                            nc.tensor.matmul(
                                dst[:, i // 2, 0:cq],
                                lhs_s[g : g + K, bc],
                                rt[g : g + K, t0 : t0 + cq],
                                tile_position=(g, 0),
                            )
                        sc = scopy.tile([128, 2, 512], f32, name="sc", tag="sc")
                        nc.scalar.copy(sc[:, :, 0:cq], pc[:, :, 0:cq])
                        # quads after the first chain their scan seed
                        # through the block's rowmin cell
                        cell = rowmin[:, ob : ob + 1]
                        nc.vector._custom_dve(
                            min2,
                            out=cell.broadcast_to((128, 2, cq)),
                            in0=pk[:, :, 0:cq],
                            in1=sc[:, :, 0:cq],
                            s0=(FP32_MAX if qi_ == 0 else cell),
                        )
                        t0 += cq

            nc.sync.dma_start(out_d[:], rowmin[:])

    nc.compile()
    return nc


def _get_program(plan):
    if plan not in _programs:
        _programs[plan] = _build_program(plan)
    return _programs[plan]


def _split16(v):
    """fp64 vector -> (hi, lo) fp16 with v ~= hi + lo to ~2^-22 rel."""
    hi = v.astype(np.float16)
    lo = (v - hi.astype(np.float64)).astype(np.float16)
    return hi.astype(np.float64), lo.astype(np.float64)


def _aug_operands(s):
    """Build [7, n] lhsT and rhs operand stacks in fp16.  Coordinates are
    fp16-rounded; the norm rows carry the hi/lo split of the ROUNDED
    points' norms, so the matmul result is the exact squared distance
    between rounded points."""
    xh = s.astype(np.float16).astype(np.float64)     # rounded coords
    n = (xh * xh).sum(axis=1)
    nh, nl = _split16(n)
    ones = np.ones(s.shape[0], dtype=np.float64)
    lhs = np.stack([xh[:, 0], xh[:, 1], xh[:, 2], nh, nl, ones, ones]).astype(
        np.float16
    )
    rhs = np.stack(
        [-2 * xh[:, 0], -2 * xh[:, 1], -2 * xh[:, 2], ones, ones, nh, nl]
    ).astype(np.float16)
    return lhs, rhs


def _rows_of(g):
    """Sorted-row slice of global rank-block g (core g%8, slot g//8)."""
    return slice(g * 128, (g + 1) * 128)


def _windows(r_rows_sorted, r_cols_sorted, plan):
    """Per global block: start of its centered (C - T_TAIL) rank-window."""
    Mc = r_cols_sorted.shape[0]
    starts = np.empty(N_BLOCKS, dtype=np.int64)
    for g in range(N_BLOCKS):
        W = plan[g // 8] - T_TAIL
        blk = r_rows_sorted[_rows_of(g)]
        a = np.searchsorted(r_cols_sorted, blk[0])
        bb = np.searchsorted(r_cols_sorted, blk[-1])
        pad = max(0, (W - (bb - a)) // 2)
        starts[g] = min(max(0, a - pad), Mc - W)
    return starts


def _margins(r_rows_sorted, r_cols_sorted, starts, plan):
    """Per-point radius-gap to the nearest EXCLUDED candidate (certificate
    bound), in sorted-row order.  Excluded ranks: [0,s) u [s+W, Mc-T)."""
    Mc = r_cols_sorted.shape[0]
    marg = np.empty(N_BLOCKS * 128, dtype=np.float64)
    for g in range(N_BLOCKS):
        W = plan[g // 8] - T_TAIL
        blk = r_rows_sorted[_rows_of(g)]
        s = int(starts[g])
        m = np.full(128, np.inf)
        if s > 0:
            m = np.minimum(m, np.maximum(blk - r_cols_sorted[s - 1], 0.0))
        if s + W < Mc - T_TAIL:
            glo = r_cols_sorted[s + W]
            ghi = r_cols_sorted[Mc - T_TAIL - 1]
            gm = np.where(blk < glo, glo - blk,
                          np.where(blk > ghi, blk - ghi, 0.0))
            m = np.minimum(m, gm)
        marg[_rows_of(g)] = m
    return marg


def _gather_rhs_group(rhs_stack, starts, plan, core, tail):
    """[4K, GW] per-group gathered candidate columns for one direction."""
    gw = [c // 4 for c in plan]
    GW = int(np.sum(gw))
    out = np.empty((4 * K, GW), dtype=np.float16)
    off = 0
    for b in range(BLOCKS):
        g = b * 8 + core
        W = plan[b] - T_TAIL
        s = int(starts[g])
        cols = np.concatenate([np.arange(s, s + W), tail])
        gs = rhs_stack[:, cols]                      # [K, C]
        parts = [[] for _ in range(4)]
        q0 = 0
        for cq in _quads(plan[b]):
            for i in range(4):
                parts[i].append(gs[:, q0 + i * cq : q0 + (i + 1) * cq])
            q0 += 4 * cq
        for i in range(4):
            out[i * K : (i + 1) * K, off : off + gw[b]] = np.concatenate(
                parts[i], axis=1
            )
        off += gw[b]
    return out


def _core_inputs(lhs1_s, rhs1_s, lhs2_s, rhs2_s, starts1, starts2, plan, core):
    """Build the four phase arrays for one core."""
    gw = [c // 4 for c in plan]
    GW = int(np.sum(gw))
    E1 = int(np.sum(gw[:1]))
    E2 = int(np.sum(gw[:4]))
    L = ROWS_PER_CORE
    Mc = rhs2_s.shape[1]
    tail = np.arange(Mc - T_TAIL, Mc)

    rows = np.concatenate(
        [np.arange((b * 8 + core) * 128, (b * 8 + core + 1) * 128)
         for b in range(BLOCKS)]
    )
    l1 = np.tile(lhs1_s[:, rows], (4, 1))           # [4K, 1024]
    l2 = np.tile(lhs2_s[:, rows], (4, 1))
    r0 = _gather_rhs_group(rhs2_s, starts1, plan, core, tail)   # dir0: set2
    r1 = _gather_rhs_group(rhs1_s, starts2, plan, core, tail)   # dir1: set1

    early = np.ascontiguousarray(
        np.concatenate([l1, r0[:, 0:E1]], axis=1)
    ).astype(np.float16)
    mid1 = np.ascontiguousarray(r0[:, E1:E2]).astype(np.float16)
    mid2 = np.ascontiguousarray(r0[:, E2:GW]).astype(np.float16)
    late = np.ascontiguousarray(np.concatenate([l2, r1], axis=1)).astype(
        np.float16
    )
    return {"early": early, "mid1": mid1, "mid2": mid2, "late": late}


def _run_plan(plan, lhs1_s, rhs1_s, lhs2_s, rhs2_s, starts1, starts2, trace=False):
    from concourse.bass_utils import run_bass_kernel_spmd

    nc = _get_program(tuple(plan))
    in_maps = [
        _core_inputs(lhs1_s, rhs1_s, lhs2_s, rhs2_s, starts1, starts2, plan, r)
        for r in range(N_CORES)
    ]

    last_err = None
    for _attempt in range(3):
        try:
            res = run_bass_kernel_spmd(nc, in_maps, list(range(N_CORES)), trace=trace)
            break
        except Exception as e:
            last_err = e
    else:
        raise last_err

    # out[:, o*8+b] on core r = sorted rows of global block g = b*8 + r
    d1 = np.empty(N, dtype=np.float32)
    d2 = np.empty(M, dtype=np.float32)
    for r in range(N_CORES):
        o = res.results[r]["out"]
        for b in range(BLOCKS):
            d1[_rows_of(b * 8 + r)] = o[:, b]
            d2[_rows_of(b * 8 + r)] = o[:, BLOCKS + b]
    return d1, d2, res


def _sample_ratio(s_rows_sorted, other, margins, step=32):
    """max over sampled rows of exact_nn_dist / certificate_margin."""
    idx = np.arange(0, s_rows_sorted.shape[0], step)
    q = s_rows_sorted[idx].astype(np.float64)
    ot = other.astype(np.float64)
    d2 = (q * q).sum(1)[:, None] + (ot * ot).sum(1)[None, :] - 2.0 * (q @ ot.T)
    d = np.sqrt(np.maximum(d2.min(axis=1), 0.0))
    m = margins[idx]
    return float(np.max(d / np.maximum(m, 1e-12)))


LAST = {}

# fp16 coordinate rounding moves each point by at most ~7e-4 (coords up to
# ~5 sigma), so device mins and true mins differ by < 2x that; the
# certificate demands this much extra headroom.
ROUND_SLACK = 2e-3


def _run_device(s1, s2, trace=False):
    """Adaptive banded run: returns ((d1, ok1), (d2, ok2), res)."""
    r1 = np.linalg.norm(s1.astype(np.float64), axis=1)
    r2 = np.linalg.norm(s2.astype(np.float64), axis=1)
    o1 = np.argsort(r1, kind="stable")
    o2 = np.argsort(r2, kind="stable")
    s1s, s2s = s1[o1], s2[o2]
    r1s, r2s = r1[o1], r2[o2]

    lhs1_s, rhs1_s = _aug_operands(s1s)
    lhs2_s, rhs2_s = _aug_operands(s2s)

    def _patch(dd, marg, rows_sorted, other):
        """Recompute exact NN for certificate-failing points (host, cheap).
        Returns (patched dd, ok)."""
        bad = np.flatnonzero(dd + ROUND_SLACK > marg)
        if bad.size == 0:
            return dd, True
        if bad.size > PATCH_QUOTA:
            return dd, False
        q = rows_sorted[bad].astype(np.float64)
        ot = other.astype(np.float64)
        d2 = (q * q).sum(1)[:, None] + (ot * ot).sum(1)[None, :] - 2.0 * (q @ ot.T)
        dd = dd.copy()
        dd[bad] = np.sqrt(np.maximum(d2.min(axis=1), 0.0))
        LAST["patched"] = LAST.get("patched", 0) + int(bad.size)
        return dd, True

    for plan in (PLAN_NARROW, PLAN_WIDE):
        starts1 = _windows(r1s, r2s, plan)
        starts2 = _windows(r2s, r1s, plan)
        marg1 = _margins(r1s, r2s, starts1, plan)
        marg2 = _margins(r2s, r1s, starts2, plan)
        if plan is PLAN_NARROW:
            # deterministic strided sample: skip the narrow run entirely if
            # exact sampled NN distances already crowd the margins
            ratio = max(
                _sample_ratio(s1s, s2, marg1), _sample_ratio(s2s, s1, marg2)
            )
            LAST["sample_ratio"] = ratio
            if ratio > 1.5:
                continue
        LAST["patched"] = 0
        d1, d2, res = _run_plan(
            plan, lhs1_s, rhs1_s, lhs2_s, rhs2_s, starts1, starts2, trace=trace
        )
        dd1 = np.sqrt(np.maximum(d1.astype(np.float64), 0.0))
        dd2 = np.sqrt(np.maximum(d2.astype(np.float64), 0.0))
        dd1, ok1 = _patch(dd1, marg1, s1s, s2)
        dd2, ok2 = _patch(dd2, marg2, s2s, s1)
        if ok1 and ok2:
            LAST["plan"] = "narrow" if plan is PLAN_NARROW else "wide"
            LAST["cert"] = True
            return (dd1, True), (dd2, True), res

    LAST["plan"] = "wide"
    LAST["cert"] = False
    return (dd1, False), (dd2, False), res


def _full_fallback(s1, s2):
    """Exact chunked numpy NN mins, used only if the certificate fails."""
    s1 = s1.astype(np.float64)
    s2 = s2.astype(np.float64)
    n1 = (s1 * s1).sum(1)
    n2 = (s2 * s2).sum(1)
    d1 = np.full(s1.shape[0], np.inf)
    d2 = np.full(s2.shape[0], np.inf)
    for i in range(0, s1.shape[0], 512):
        blk = n1[i : i + 512, None] + n2[None, :] - 2.0 * (s1[i : i + 512] @ s2.T)
        np.maximum(blk, 0.0, out=blk)
        d1[i : i + 512] = blk.min(axis=1)
        d2 = np.minimum(d2, blk.min(axis=0))
    return np.sqrt(d1), np.sqrt(d2)


def kernel(set1, set2, hausdorff=0, w_set1_set2=1, w_set2_set1=1, n_outputs=1):
    s1 = np.ascontiguousarray(np.asarray(set1, dtype=np.float32))
    s2 = np.ascontiguousarray(np.asarray(set2, dtype=np.float32))
    assert s1.shape == (N, D) and s2.shape == (M, D), (s1.shape, s2.shape)
    hausdorff = int(np.asarray(hausdorff))
    w12 = int(np.asarray(w_set1_set2))
    w21 = int(np.asarray(w_set2_set1))
    n_outputs = int(np.asarray(n_outputs))

    (d1, ok1), (d2, ok2), _ = _run_device(s1, s2)
    if not (ok1 and ok2):
        d1, d2 = _full_fallback(s1, s2)

    reduce = np.mean if hausdorff == 0 else np.max
    t12 = np.float32(reduce(d1)) if w12 != 0 else np.float32(0.0)
    t21 = np.float32(reduce(d2)) if w21 != 0 else np.float32(0.0)

    if n_outputs == 1:
        return np.float32(t12 + t21)
    return (t12, t21)
